# revision 42
# baseline (speedup 1.0000x reference)
"""Trainium2 Bass kernel for causal self-attention (nn_CausalSelfAttention).

Problem (hardcoded):
    x:     [1, 4096, 1024] f32
    w_qkv: [1024, 3072] f32, b_qkv: [3072] f32
    w_out: [1024, 1024] f32, b_out: [1024] f32
    16 heads, head_dim 64, causal softmax attention.

Sharding: tensor-parallel over heads. 8 cores x 2 heads each. Each core
computes QKV for its heads, T^2 causal attention, and a partial output
projection; host sums the 8 partial projections (the all-reduce) and adds
biases.

Math notes (exact simplifications):
  - b_k drops out: S[t,s] += q_t . b_k is constant per query row; softmax is
    shift-invariant along s.
  - b_v reduces to a host-side constant: O_h = sum_s a_s (v_s + b_v_h)
    = sum_s a_s v_s + b_v_h (attention weights sum to 1), so its contribution
    to the output is the constant row b_v @ w_out.
  - b_q is applied on-device as a per-partition bias when copying Q^T out of
    PSUM (free).
  - Per-token softmax denominators commute with the per-head output
    projection, so we normalize O per head right before the projection.

Numerics: x, w_qkv, q, k, v, exp(scores), w_out run in bf16 (the two K=64
score matmuls then row-tile into PE row groups 0/64 and run concurrently,
and FWL accelerates the weight loads); PSUM accumulation is f32 throughout;
partial y is written in fp16 and summed on the host in f32. Measured rms
error ~4e-3 vs the f32 reference (tolerance 2e-2).

Device layout (per core, SPMD; all 8 cores run the same program):
  - x^T is precomputed host-side as bf16 [1024, 4096] so all QKV matmuls can
    stream it directly (contraction dim on partitions).
  - Q^T, K^T: [128 (2 heads x 64 dim), T] bf16 tiles; V': [T, 2x(64+1)]
    bf16 with a ones column appended per head so the attention-value
    matmul also produces the softmax denominator in partition 64.
  - Scores are computed transposed (S^T [keys, queries]) so the softmax
    reduction over keys is the matmul contraction, never a partition-axis
    reduction, and exp(S^T) feeds the AV matmul directly with no transposes.
  - Causal masking: tk-chunks strictly above the diagonal are skipped; the
    single partial 128x128 block per diagonal chunk is masked by multiplying
    exp by an upper-triangular 0/1 mask (exp of the skipped columns is never
    computed: matmul N-ranges shrink on diagonal chunks).
"""

import os

import numpy as np
import ml_dtypes

T = 4096
E = 1024
NCORES = 8
D = 64  # head dim
TQ = 512  # query tile (8 tiles)
NJ = T // TQ

_CACHE = {}

# Results of the last SPMD run (exec_time_ns etc.), for the local test harness.
LAST_RESULTS = None


def _build():
    import concourse.bacc as bacc
    import concourse.tile as tile
    import concourse.mybir as mybir

    f32 = mybir.dt.float32
    f32r = mybir.dt.float32r
    bf16 = mybir.dt.bfloat16
    f16 = mybir.dt.float16
    EXP = mybir.ActivationFunctionType.Exp

    nc = bacc.Bacc("TRN2", target_bir_lowering=False, debug=False)

    xT = nc.dram_tensor("xT", [E, T], bf16, kind="ExternalInput").ap()
    # per-core slice of w_qkv: cols [q(128) | k(128) | v(128)] for this core's
    # two heads
    wqkv = nc.dram_tensor("wqkv", [E, 384], bf16, kind="ExternalInput").ap()
    bq = nc.dram_tensor("bq", [128], f32, kind="ExternalInput").ap()
    wo = nc.dram_tensor("wo", [128, E], bf16, kind="ExternalInput").ap()
    # selector rows at partitions 0 and 32 (engine APs need 32-aligned bases):
    # row0 -> head0 partitions, row32 -> head1 partitions
    sel_dram = nc.dram_tensor("sel", [33, 128], bf16, kind="ExternalInput").ap()
    mask_dram = nc.dram_tensor("mask", [128, 128], bf16, kind="ExternalInput").ap()
    ident_dram = nc.dram_tensor("ident", [128, 128], bf16, kind="ExternalInput").ap()
    y = nc.dram_tensor("y", [T, E], f16, kind="ExternalOutput").ap()

    with tile.TileContext(nc) as tc:
        with (
            tc.tile_pool(name="consts", bufs=1) as consts,
            tc.tile_pool(name="w", bufs=8) as wpool,
            tc.tile_pool(name="xt", bufs=32) as xtp,
            tc.tile_pool(name="qt", bufs=NJ) as qtp,
            tc.tile_pool(name="kt", bufs=NJ) as ktp,
            tc.tile_pool(name="v", bufs=NJ) as vp,
            tc.tile_pool(name="vts", bufs=2) as vtsp,
            tc.tile_pool(name="expst", bufs=6) as exp_p,
            tc.tile_pool(name="otn", bufs=2) as otnp,
            tc.tile_pool(name="bb", bufs=2) as bbp,
            tc.tile_pool(name="rd", bufs=4) as rdp,
            tc.tile_pool(name="ysb", bufs=3) as ysp,
            tc.tile_pool(name="mm_ps", bufs=2, space="PSUM") as mmp,
            tc.tile_pool(name="st_ps", bufs=2, space="PSUM") as stp,
            tc.tile_pool(name="op_ps", bufs=2, space="PSUM") as opp,
        ):
            # weights + the first token block's x^T tiles are interleaved so
            # the j=0 QKV matmuls can start as soon as the first pair lands;
            # constants (needed later) load after.
            w_sb = []
            xts0 = []
            bq_sb = consts.tile([128, 1], f32)
            for e in range(8):
                w = wpool.tile([128, 384], bf16)
                nc.sync.dma_start(w[:], wqkv[128 * e : 128 * (e + 1), :])
                w_sb.append(w)
                xt = xtp.tile([128, TQ], bf16)
                nc.sync.dma_start(xt[:], xT[128 * e : 128 * (e + 1), 0:TQ])
                xts0.append(xt)
                if e == 0:
                    nc.sync.dma_start(bq_sb[:, 0], bq[:])
            ident = consts.tile([128, 128], bf16)
            nc.sync.dma_start(ident[:], ident_dram[:])
            mask = consts.tile([128, 128], bf16)  # 1 where tq >= tk else 0
            nc.sync.dma_start(mask[:], mask_dram[:])
            sel = consts.tile([33, 128], bf16)
            nc.sync.dma_start(sel[:], sel_dram[:])
            wo_sb = consts.tile([128, E], bf16)
            nc.sync.dma_start(wo_sb[:], wo[:])

            # warm up the PE's HAM clock gate during the initial DMA wait:
            # ~3.5us of dummy matmuls on a memset tile so the first real
            # matmuls run at 2.4 GHz instead of 1.2
            warm = consts.tile([128, 64], bf16)
            nc.vector.memset(warm[:], 0.0)
            wps = mmp.tile([64, 64], f32, tag="mm")
            NWARM = 56
            for i in range(NWARM):
                nc.tensor.matmul(
                    wps[:], warm[:, 0:64], warm[:],
                    start=(i == 0), stop=(i == NWARM - 1),
                )

            def emit_outproj(otn, t0, tail=False):
                # partial output projection for the tile whose normalized
                # O^T is `otn` (tokens [t0, t0+TQ)). In tail mode (last tile,
                # scalar engine idle) the PSUM->SBUF casts alternate between
                # the vector and scalar engines to shorten the serial tail.
                for c in range(4):
                    ys = ysp.tile([128, E], f16, tag="ys", name=f"ys_{t0}_{c}")
                    for half in range(2):
                        yp = mmp.tile([128, 512], f32, tag="mm", name=f"yp_{t0}_{c}_{half}")
                        nc.tensor.matmul(
                            yp[:],
                            otn[:, 128 * c : 128 * (c + 1)],
                            wo_sb[:, 512 * half : 512 * (half + 1)],
                            start=True, stop=True,
                        )
                        if tail and half == 1:
                            nc.scalar.copy(
                                ys[:, 512 * half : 512 * (half + 1)], yp[:]
                            )
                        else:
                            nc.vector.tensor_copy(
                                ys[:, 512 * half : 512 * (half + 1)], yp[:]
                            )
                    nc.sync.dma_start(
                        y[t0 + 128 * c : t0 + 128 * (c + 1), :], ys[:]
                    )

            def norm_chain(pend):
                # finish the pending tile's normalization: broadcast 1/denom
                # to the head partitions via K=1 matmuls (row-tiled at 0/32),
                # then scale O'
                ops, rdpack, t0 = pend
                bps = stp.tile([128, TQ], f32, tag="st", name=f"bps_{t0}")
                nc.tensor.matmul(
                    bps[:], sel[0:1, 0:128], rdpack[0:1, :],
                    start=True, stop=False,
                )
                nc.tensor.matmul(
                    bps[:], sel[32:33, 0:128], rdpack[32:33, :],
                    start=False, stop=True,
                )
                bb = bbp.tile([128, TQ], f32, tag="bb", name=f"bb_{t0}")
                nc.vector.reciprocal_approx_fast(bb[:], bps[:])
                otn = otnp.tile([128, TQ], bf16, tag="otn", name=f"otn_{t0}")
                nc.vector.tensor_mul(otn[0:64, :], ops[0][0:64, :], bb[0:64, :])
                nc.vector.tensor_mul(otn[64:128, :], ops[1][0:64, :], bb[64:128, :])
                return otn, t0

            pending = None
            prev_otn = None
            kt_tiles = []
            v_tiles = []

            def emit_q(xts):
                ps_q = mmp.tile([128, TQ], f32, tag="mm")
                for e in range(8):
                    nc.tensor.matmul(
                        ps_q[:], w_sb[e][:, 0:128], xts[e][:],
                        start=(e == 0), stop=(e == 7),
                    )
                qt = qtp.tile([128, TQ], bf16)
                # fold b_q in as a per-partition bias
                nc.vector.tensor_scalar_add(qt[:], ps_q[:], bq_sb[:, 0:1])
                return qt

            def emit_k(xts):
                ps_k = mmp.tile([128, TQ], f32, tag="mm")
                for e in range(8):
                    nc.tensor.matmul(
                        ps_k[:], w_sb[e][:, 128:256], xts[e][:],
                        start=(e == 0), stop=(e == 7),
                    )
                kt = ktp.tile([128, TQ], bf16)
                nc.vector.tensor_copy(kt[:], ps_k[:])
                kt_tiles.append(kt)

            def emit_v(xts):
                ps_vt = mmp.tile([128, TQ], f32, tag="mm")
                for e in range(8):
                    nc.tensor.matmul(
                        ps_vt[:], w_sb[e][:, 256:384], xts[e][:],
                        start=(e == 0), stop=(e == 7),
                    )
                vts = vtsp.tile([128, TQ], bf16)
                nc.vector.tensor_copy(vts[:], ps_vt[:])
                # transpose V^T -> V [tokens, vfeat]; per 128-token chunk c
                # the layout is [V_h0(64) | 1 | V_h1(64) | 1]
                vt = vp.tile([128, 4 * 130], bf16)
                nc.vector.memset(
                    vt.rearrange("p (c w) -> p c w", w=130)[:, :, 64::65],
                    1.0,
                )
                ps_tr = mmp.tile([128, 512], bf16, tag="mm")
                for c in range(4):
                    nc.tensor.transpose(
                        ps_tr[:, 128 * c : 128 * (c + 1)],
                        vts[:, 128 * c : 128 * (c + 1)],
                        ident[:],
                    )
                nc.vector.tensor_copy(
                    vt.rearrange("p (c s w) -> p c s w", s=2, w=65)[:, :, :, 0:64],
                    ps_tr.rearrange("p (c s w) -> p c s w", s=2, w=64),
                )
                v_tiles.append(vt)

            # Q for tile j is computed during tile j-1's chunk loop, and for
            # j>=1 the K/V/transpose chains are woven into the first chunks
            # (off-diagonal chunks only read OLD kt/v tiles), so the scalar
            # engine's exp stream restarts almost immediately at each tile
            # boundary instead of idling behind the full QKV block.
            xts_cur = xts0
            qt = None
            for j in range(NJ):
                t0 = TQ * j
                # ---- prefetch next token block's x^T tiles ----
                if j + 1 < NJ:
                    xts_next = []
                    for e in range(8):
                        xt = xtp.tile([128, TQ], bf16)
                        nc.sync.dma_start(
                            xt[:],
                            xT[128 * e : 128 * (e + 1), t0 + TQ : t0 + 2 * TQ],
                        )
                        xts_next.append(xt)
                else:
                    xts_next = None

                if j == 0:
                    qt = emit_q(xts_cur)
                    emit_k(xts_cur)
                    emit_v(xts_cur)

                # ---- causal attention for query tile j (both heads) ----
                op0 = opp.tile([65, TQ], f32, tag="op")
                op1 = opp.tile([65, TQ], f32, tag="op")
                ops = [op0, op1]
                nchunks = 4 * j + 4

                def chunk_geom(g):
                    jj, c = divmod(g, 4)
                    r = g - 4 * j  # >= 0 on the block-diagonal
                    col0 = 128 * r if r >= 0 else 0
                    return jj, c, r, col0

                def emit_scores(g):
                    jj, c, r, col0 = chunk_geom(g)
                    # both heads' scores in one [128, 1024] PSUM tile; the
                    # K=64 head matmuls run concurrently (row groups 0/64)
                    st = stp.tile([128, 2 * TQ], f32, tag="st")
                    for h in range(2):
                        nc.tensor.matmul(
                            st[:, TQ * h + col0 : TQ * h + TQ],
                            kt_tiles[jj][64 * h : 64 * h + 64, 128 * c : 128 * (c + 1)],
                            qt[64 * h : 64 * h + 64, col0:TQ],
                            start=True, stop=True,
                        )
                    return st

                # scores are emitted one chunk ahead of the exp/AV for that
                # chunk: the PE drains its queue in order, so AV(g) (gated on
                # the exp) must not sit ahead of scores(g+1) (not gated)
                st_cur = emit_scores(0)
                for g in range(nchunks):
                    jj, c, r, col0 = chunk_geom(g)
                    ex = exp_p.tile([128, 2 * TQ], bf16, tag="ex")
                    st3 = st_cur.rearrange("p (h n) -> p h n", h=2)
                    ex3 = ex.rearrange("p (h n) -> p h n", h=2)
                    nc.scalar.activation(
                        ex3[:, :, col0:TQ], st3[:, :, col0:TQ], EXP, scale=0.125
                    )
                    if g + 1 < nchunks:
                        st_cur = emit_scores(g + 1)
                    if r >= 0:
                        for h in range(2):
                            nc.vector.tensor_mul(
                                ex[:, TQ * h + col0 : TQ * h + col0 + 128],
                                ex[:, TQ * h + col0 : TQ * h + col0 + 128],
                                mask[:],
                            )
                    for h in range(2):
                        nc.tensor.matmul(
                            ops[h][:, col0:TQ],
                            v_tiles[jj][:, 130 * c + 65 * h : 130 * c + 65 * h + 65],
                            ex[:, TQ * h + col0 : TQ * h + TQ],
                            start=(g == 0), stop=(g == nchunks - 1),
                            skip_group_check=True,
                        )
                    if j > 0:
                        # weave this tile's K/V chains into the first chunks
                        # (needed only from the diagonal chunks, g >= 4j >= 4)
                        if g == 0:
                            emit_k(xts_cur)
                        elif g == 1:
                            emit_v(xts_cur)
                    if g == nchunks // 2 and xts_next is not None:
                        qt_next = emit_q(xts_next)

                # grab the two denominator rows as soon as the AV
                # accumulation finishes; the rest of the normalization is
                # deferred into the next iteration (after its QKV matmuls)
                rdpack = rdp.tile([33, TQ], bf16, tag="rd")
                with nc.allow_low_precision(reason="bf16 rounding of denom"):
                    nc.vector.tensor_copy(rdpack[0:1, :], ops[0][64:65, :])
                    nc.vector.tensor_copy(rdpack[32:33, :], ops[1][64:65, :])
                # emit the previous tile's output projection here: its (ready)
                # matmuls keep the PE busy while the reciprocal chain runs
                if prev_otn is not None:
                    emit_outproj(*prev_otn)
                prev_otn = norm_chain((ops, rdpack, t0))
                xts_cur = xts_next
                if xts_next is not None:
                    qt = qt_next

            emit_outproj(*prev_otn, tail=True)

    nc.compile()
    return nc


def _prep_inputs(x, w_qkv, b_qkv, w_out, b_out):
    x = np.asarray(x, dtype=np.float32).reshape(T, E)
    w_qkv = np.asarray(w_qkv, dtype=np.float32)
    b_qkv = np.asarray(b_qkv, dtype=np.float32)
    w_out = np.asarray(w_out, dtype=np.float32)
    b_out = np.asarray(b_out, dtype=np.float32)

    xT = np.ascontiguousarray(x.T).astype(ml_dtypes.bfloat16)
    mask = np.triu(np.ones((128, 128), dtype=np.float32)).astype(ml_dtypes.bfloat16)
    ident = np.eye(128, dtype=np.float32).astype(ml_dtypes.bfloat16)
    sel = np.zeros((33, 128), dtype=np.float32)
    sel[0, 0:64] = 1.0
    sel[32, 64:128] = 1.0
    sel = sel.astype(ml_dtypes.bfloat16)

    in_maps = []
    for cidx in range(NCORES):
        lo, hi = 128 * cidx, 128 * (cidx + 1)
        wq = w_qkv[:, lo:hi]
        wk = w_qkv[:, E + lo : E + hi]
        wv = w_qkv[:, 2 * E + lo : 2 * E + hi]
        wqkv_c = np.ascontiguousarray(
            np.concatenate([wq, wk, wv], axis=1)
        ).astype(ml_dtypes.bfloat16)
        in_maps.append(
            {
                "xT": xT,
                "wqkv": wqkv_c,
                "bq": np.ascontiguousarray(b_qkv[lo:hi]),
                "wo": np.ascontiguousarray(w_out[lo:hi, :]).astype(
                    ml_dtypes.bfloat16
                ),
                "sel": sel,
                "mask": mask,
                "ident": ident,
            }
        )
    # host-side constant: b_out plus the exact b_v contribution
    b_v = b_qkv[2 * E : 3 * E]
    const_row = b_out + b_v @ w_out
    return in_maps, const_row


def kernel(x, w_qkv, b_qkv, w_out, b_out):
    global LAST_RESULTS
    from concourse.bass_utils import run_bass_kernel_spmd

    if "nc" not in _CACHE:
        _CACHE["nc"] = _build()
    nc = _CACHE["nc"]

    in_maps, const_row = _prep_inputs(x, w_qkv, b_qkv, w_out, b_out)
    res = run_bass_kernel_spmd(nc, in_maps, core_ids=list(range(NCORES)))
    LAST_RESULTS = res

    out = np.zeros((T, E), dtype=np.float32)
    for r in res.results:
        out += np.asarray(r["y"], dtype=np.float32)
    out += const_row[None, :].astype(np.float32)
    return out.reshape(1, T, E)



# revision 45
# speedup vs baseline: 1.0249x; 1.0249x over previous
"""Trainium2 Bass kernel for causal self-attention (nn_CausalSelfAttention).

Problem (hardcoded):
    x:     [1, 4096, 1024] f32
    w_qkv: [1024, 3072] f32, b_qkv: [3072] f32
    w_out: [1024, 1024] f32, b_out: [1024] f32
    16 heads, head_dim 64, causal softmax attention.

Sharding: tensor-parallel over heads. 8 cores x 2 heads each. Each core
computes QKV for its heads, T^2 causal attention, and a partial output
projection; host sums the 8 partial projections (the all-reduce) and adds
biases.

Math notes (exact simplifications):
  - b_k drops out: S[t,s] += q_t . b_k is constant per query row; softmax is
    shift-invariant along s.
  - b_v reduces to a host-side constant: O_h = sum_s a_s (v_s + b_v_h)
    = sum_s a_s v_s + b_v_h (attention weights sum to 1), so its contribution
    to the output is the constant row b_v @ w_out.
  - b_q is applied on-device as a per-partition bias when copying Q^T out of
    PSUM (free).
  - Per-token softmax denominators commute with the per-head output
    projection, so we normalize O per head right before the projection.

Numerics: x, w_qkv, q, k, v, exp(scores), w_out run in bf16 (the two K=64
score matmuls then row-tile into PE row groups 0/64 and run concurrently,
and FWL accelerates the weight loads); PSUM accumulation is f32 throughout;
partial y is written in fp16 and summed on the host in f32. Measured rms
error ~4e-3 vs the f32 reference (tolerance 2e-2).

Device layout (per core, SPMD; all 8 cores run the same program):
  - x^T is precomputed host-side as bf16 [1024, 4096] so all QKV matmuls can
    stream it directly (contraction dim on partitions).
  - Q^T, K^T: [128 (2 heads x 64 dim), T] bf16 tiles; V': [T, 2x(64+1)]
    bf16 with a ones column appended per head so the attention-value
    matmul also produces the softmax denominator in partition 64.
  - Scores are computed transposed (S^T [keys, queries]) so the softmax
    reduction over keys is the matmul contraction, never a partition-axis
    reduction, and exp(S^T) feeds the AV matmul directly with no transposes.
  - Causal masking: tk-chunks strictly above the diagonal are skipped; the
    single partial 128x128 block per diagonal chunk is masked by multiplying
    exp by an upper-triangular 0/1 mask (exp of the skipped columns is never
    computed: matmul N-ranges shrink on diagonal chunks).
"""

import os

import numpy as np
import ml_dtypes

T = 4096
E = 1024
NCORES = 8
D = 64  # head dim
TQ = 512  # query tile (8 tiles)
NJ = T // TQ

_CACHE = {}

# Results of the last SPMD run (exec_time_ns etc.), for the local test harness.
LAST_RESULTS = None


def _build():
    import concourse.bacc as bacc
    import concourse.tile as tile
    import concourse.mybir as mybir

    f32 = mybir.dt.float32
    f32r = mybir.dt.float32r
    bf16 = mybir.dt.bfloat16
    f16 = mybir.dt.float16
    EXP = mybir.ActivationFunctionType.Exp

    nc = bacc.Bacc("TRN2", target_bir_lowering=False, debug=False)

    xT = nc.dram_tensor("xT", [E, T], bf16, kind="ExternalInput").ap()
    # per-core slice of w_qkv: cols [q(128) | k(128) | v(128)] for this core's
    # two heads
    wqkv = nc.dram_tensor("wqkv", [E, 384], bf16, kind="ExternalInput").ap()
    bq = nc.dram_tensor("bq", [128], f32, kind="ExternalInput").ap()
    wo = nc.dram_tensor("wo", [128, E], bf16, kind="ExternalInput").ap()
    # selector rows at partitions 0 and 32 (engine APs need 32-aligned bases):
    # row0 -> head0 partitions, row32 -> head1 partitions
    sel_dram = nc.dram_tensor("sel", [33, 128], bf16, kind="ExternalInput").ap()
    mask_dram = nc.dram_tensor("mask", [128, 128], bf16, kind="ExternalInput").ap()
    ident_dram = nc.dram_tensor("ident", [128, 128], bf16, kind="ExternalInput").ap()
    y = nc.dram_tensor("y", [T, E], f16, kind="ExternalOutput").ap()

    with tile.TileContext(nc) as tc:
        with (
            tc.tile_pool(name="consts", bufs=1) as consts,
            tc.tile_pool(name="w", bufs=8) as wpool,
            tc.tile_pool(name="xt", bufs=32) as xtp,
            tc.tile_pool(name="qt", bufs=NJ) as qtp,
            tc.tile_pool(name="kt", bufs=NJ) as ktp,
            tc.tile_pool(name="v", bufs=NJ) as vp,
            tc.tile_pool(name="vts", bufs=2) as vtsp,
            tc.tile_pool(name="expst", bufs=6) as exp_p,
            tc.tile_pool(name="otn", bufs=2) as otnp,
            tc.tile_pool(name="bb", bufs=2) as bbp,
            tc.tile_pool(name="rd", bufs=4) as rdp,
            tc.tile_pool(name="ysb", bufs=3) as ysp,
            tc.tile_pool(name="mm_ps", bufs=2, space="PSUM") as mmp,
            tc.tile_pool(name="st_ps", bufs=2, space="PSUM") as stp,
            tc.tile_pool(name="op_ps", bufs=2, space="PSUM") as opp,
        ):
            # weights + the first token block's x^T tiles are interleaved so
            # the j=0 QKV matmuls can start as soon as the first pair lands;
            # constants (needed later) load after.
            w_sb = []
            xts0 = []
            bq_sb = consts.tile([128, 1], f32)
            for e in range(8):
                w = wpool.tile([128, 384], bf16)
                nc.sync.dma_start(w[:], wqkv[128 * e : 128 * (e + 1), :])
                w_sb.append(w)
                xt = xtp.tile([128, TQ], bf16)
                nc.sync.dma_start(xt[:], xT[128 * e : 128 * (e + 1), 0:TQ])
                xts0.append(xt)
                if e == 0:
                    nc.sync.dma_start(bq_sb[:, 0], bq[:])
            ident = consts.tile([128, 128], bf16)
            nc.sync.dma_start(ident[:], ident_dram[:])
            mask = consts.tile([128, 128], bf16)  # 1 where tq >= tk else 0
            nc.sync.dma_start(mask[:], mask_dram[:])
            sel = consts.tile([33, 128], bf16)
            nc.sync.dma_start(sel[:], sel_dram[:])
            wo_sb = consts.tile([128, E], bf16)
            nc.sync.dma_start(wo_sb[:], wo[:])

            # warm up the PE's HAM clock gate during the initial DMA wait:
            # ~3.5us of dummy matmuls on a memset tile so the first real
            # matmuls run at 2.4 GHz instead of 1.2
            warm = consts.tile([128, 64], bf16)
            nc.vector.memset(warm[:], 0.0)
            wps = mmp.tile([64, 64], f32, tag="mm")
            NWARM = 56
            for i in range(NWARM):
                nc.tensor.matmul(
                    wps[:], warm[:, 0:64], warm[:],
                    start=(i == 0), stop=(i == NWARM - 1),
                )

            def emit_outproj(otn, t0, tail=False):
                # partial output projection for the tile whose normalized
                # O^T is `otn` (tokens [t0, t0+TQ)). In tail mode (last tile,
                # scalar engine idle) the PSUM->SBUF casts alternate between
                # the vector and scalar engines to shorten the serial tail.
                for c in range(4):
                    ys = ysp.tile([128, E], f16, tag="ys", name=f"ys_{t0}_{c}")
                    for half in range(2):
                        yp = mmp.tile([128, 512], f32, tag="mm", name=f"yp_{t0}_{c}_{half}")
                        nc.tensor.matmul(
                            yp[:],
                            otn[:, 128 * c : 128 * (c + 1)],
                            wo_sb[:, 512 * half : 512 * (half + 1)],
                            start=True, stop=True,
                        )
                        if tail and half == 1:
                            nc.scalar.copy(
                                ys[:, 512 * half : 512 * (half + 1)], yp[:]
                            )
                        else:
                            nc.vector.tensor_copy(
                                ys[:, 512 * half : 512 * (half + 1)], yp[:]
                            )
                    nc.sync.dma_start(
                        y[t0 + 128 * c : t0 + 128 * (c + 1), :], ys[:]
                    )

            def norm_chain(pend):
                # finish the pending tile's normalization: broadcast 1/denom
                # to the head partitions via K=1 matmuls (row-tiled at 0/32),
                # then scale O'
                ops, rdpack, t0 = pend
                bps = stp.tile([128, TQ], f32, tag="st", name=f"bps_{t0}")
                nc.tensor.matmul(
                    bps[:], sel[0:1, 0:128], rdpack[0:1, :],
                    start=True, stop=False,
                )
                nc.tensor.matmul(
                    bps[:], sel[32:33, 0:128], rdpack[32:33, :],
                    start=False, stop=True,
                )
                bb = bbp.tile([128, TQ], f32, tag="bb", name=f"bb_{t0}")
                nc.vector.reciprocal_approx_fast(bb[:], bps[:])
                otn = otnp.tile([128, TQ], bf16, tag="otn", name=f"otn_{t0}")
                nc.vector.tensor_mul(otn[0:64, :], ops[0][0:64, :], bb[0:64, :])
                nc.vector.tensor_mul(otn[64:128, :], ops[1][0:64, :], bb[64:128, :])
                return otn, t0

            pending = None
            prev_otn = None
            kt_tiles = []
            v_tiles = []

            def emit_q(xts):
                ps_q = mmp.tile([128, TQ], f32, tag="mm")
                for e in range(8):
                    nc.tensor.matmul(
                        ps_q[:], w_sb[e][:, 0:128], xts[e][:],
                        start=(e == 0), stop=(e == 7),
                    )
                qt = qtp.tile([128, TQ], bf16)
                # fold b_q in as a per-partition bias
                nc.vector.tensor_scalar_add(qt[:], ps_q[:], bq_sb[:, 0:1])
                return qt

            def emit_k(xts):
                ps_k = mmp.tile([128, TQ], f32, tag="mm")
                for e in range(8):
                    nc.tensor.matmul(
                        ps_k[:], w_sb[e][:, 128:256], xts[e][:],
                        start=(e == 0), stop=(e == 7),
                    )
                kt = ktp.tile([128, TQ], bf16)
                nc.vector.tensor_copy(kt[:], ps_k[:])
                kt_tiles.append(kt)

            def emit_v(xts):
                ps_vt = mmp.tile([128, TQ], f32, tag="mm")
                for e in range(8):
                    nc.tensor.matmul(
                        ps_vt[:], w_sb[e][:, 256:384], xts[e][:],
                        start=(e == 0), stop=(e == 7),
                    )
                vts = vtsp.tile([128, TQ], bf16)
                nc.vector.tensor_copy(vts[:], ps_vt[:])
                # transpose V^T -> V [tokens, vfeat]; per 128-token chunk c
                # the layout is [V_h0(64) | 1 | V_h1(64) | 1]
                vt = vp.tile([128, 4 * 130], bf16)
                nc.vector.memset(
                    vt.rearrange("p (c w) -> p c w", w=130)[:, :, 64::65],
                    1.0,
                )
                ps_tr = mmp.tile([128, 512], bf16, tag="mm")
                for c in range(4):
                    nc.tensor.transpose(
                        ps_tr[:, 128 * c : 128 * (c + 1)],
                        vts[:, 128 * c : 128 * (c + 1)],
                        ident[:],
                    )
                nc.vector.tensor_copy(
                    vt.rearrange("p (c s w) -> p c s w", s=2, w=65)[:, :, :, 0:64],
                    ps_tr.rearrange("p (c s w) -> p c s w", s=2, w=64),
                )
                v_tiles.append(vt)

            for j in range(NJ):
                t0 = TQ * j
                # ---- load x^T tiles for this token block ----
                if j == 0:
                    xts = xts0
                else:
                    xts = []
                    for e in range(8):
                        xt = xtp.tile([128, TQ], bf16)
                        nc.sync.dma_start(
                            xt[:], xT[128 * e : 128 * (e + 1), t0 : t0 + TQ]
                        )
                        xts.append(xt)

                # ---- Q^T, K^T, V^T via w-stationary matmuls ----
                qt = emit_q(xts)
                emit_k(xts)
                emit_v(xts)

                # ---- causal attention for query tile j (both heads) ----
                op0 = opp.tile([65, TQ], f32, tag="op")
                op1 = opp.tile([65, TQ], f32, tag="op")
                ops = [op0, op1]
                nchunks = 4 * j + 4

                def chunk_geom(g):
                    jj, c = divmod(g, 4)
                    r = g - 4 * j  # >= 0 on the block-diagonal
                    col0 = 128 * r if r >= 0 else 0
                    return jj, c, r, col0

                def emit_scores(g):
                    jj, c, r, col0 = chunk_geom(g)
                    # both heads' scores in one [128, 1024] PSUM tile; the
                    # K=64 head matmuls run concurrently (row groups 0/64)
                    st = stp.tile([128, 2 * TQ], f32, tag="st")
                    for h in range(2):
                        nc.tensor.matmul(
                            st[:, TQ * h + col0 : TQ * h + TQ],
                            kt_tiles[jj][64 * h : 64 * h + 64, 128 * c : 128 * (c + 1)],
                            qt[64 * h : 64 * h + 64, col0:TQ],
                            start=True, stop=True,
                        )
                    return st

                # scores are emitted one chunk ahead of the exp/AV for that
                # chunk: the PE drains its queue in order, so AV(g) (gated on
                # the exp) must not sit ahead of scores(g+1) (not gated)
                st_cur = emit_scores(0)
                for g in range(nchunks):
                    jj, c, r, col0 = chunk_geom(g)
                    ex = exp_p.tile([128, 2 * TQ], bf16, tag="ex")
                    st3 = st_cur.rearrange("p (h n) -> p h n", h=2)
                    ex3 = ex.rearrange("p (h n) -> p h n", h=2)
                    nc.scalar.activation(
                        ex3[:, :, col0:TQ], st3[:, :, col0:TQ], EXP, scale=0.125
                    )
                    if g + 1 < nchunks:
                        st_cur = emit_scores(g + 1)
                    if r >= 0:
                        for h in range(2):
                            nc.vector.tensor_mul(
                                ex[:, TQ * h + col0 : TQ * h + col0 + 128],
                                ex[:, TQ * h + col0 : TQ * h + col0 + 128],
                                mask[:],
                            )
                    for h in range(2):
                        nc.tensor.matmul(
                            ops[h][:, col0:TQ],
                            v_tiles[jj][:, 130 * c + 65 * h : 130 * c + 65 * h + 65],
                            ex[:, TQ * h + col0 : TQ * h + TQ],
                            start=(g == 0), stop=(g == nchunks - 1),
                            skip_group_check=True,
                        )

                # grab the two denominator rows as soon as the AV
                # accumulation finishes; the rest of the normalization is
                # deferred into the next iteration (after its QKV matmuls)
                rdpack = rdp.tile([33, TQ], bf16, tag="rd")
                with nc.allow_low_precision(reason="bf16 rounding of denom"):
                    nc.vector.tensor_copy(rdpack[0:1, :], ops[0][64:65, :])
                    nc.vector.tensor_copy(rdpack[32:33, :], ops[1][64:65, :])
                # emit the previous tile's output projection here: its (ready)
                # matmuls keep the PE busy while the reciprocal chain runs
                if prev_otn is not None:
                    emit_outproj(*prev_otn)
                prev_otn = norm_chain((ops, rdpack, t0))

            emit_outproj(*prev_otn, tail=True)

    nc.compile()
    return nc


def _prep_inputs(x, w_qkv, b_qkv, w_out, b_out):
    x = np.asarray(x, dtype=np.float32).reshape(T, E)
    w_qkv = np.asarray(w_qkv, dtype=np.float32)
    b_qkv = np.asarray(b_qkv, dtype=np.float32)
    w_out = np.asarray(w_out, dtype=np.float32)
    b_out = np.asarray(b_out, dtype=np.float32)

    xT = np.ascontiguousarray(x.T).astype(ml_dtypes.bfloat16)
    mask = np.triu(np.ones((128, 128), dtype=np.float32)).astype(ml_dtypes.bfloat16)
    ident = np.eye(128, dtype=np.float32).astype(ml_dtypes.bfloat16)
    sel = np.zeros((33, 128), dtype=np.float32)
    sel[0, 0:64] = 1.0
    sel[32, 64:128] = 1.0
    sel = sel.astype(ml_dtypes.bfloat16)

    in_maps = []
    for cidx in range(NCORES):
        lo, hi = 128 * cidx, 128 * (cidx + 1)
        wq = w_qkv[:, lo:hi]
        wk = w_qkv[:, E + lo : E + hi]
        wv = w_qkv[:, 2 * E + lo : 2 * E + hi]
        wqkv_c = np.ascontiguousarray(
            np.concatenate([wq, wk, wv], axis=1)
        ).astype(ml_dtypes.bfloat16)
        in_maps.append(
            {
                "xT": xT,
                "wqkv": wqkv_c,
                "bq": np.ascontiguousarray(b_qkv[lo:hi]),
                "wo": np.ascontiguousarray(w_out[lo:hi, :]).astype(
                    ml_dtypes.bfloat16
                ),
                "sel": sel,
                "mask": mask,
                "ident": ident,
            }
        )
    # host-side constant: b_out plus the exact b_v contribution
    b_v = b_qkv[2 * E : 3 * E]
    const_row = b_out + b_v @ w_out
    return in_maps, const_row


def kernel(x, w_qkv, b_qkv, w_out, b_out):
    global LAST_RESULTS
    from concourse.bass_utils import run_bass_kernel_spmd

    if "nc" not in _CACHE:
        _CACHE["nc"] = _build()
    nc = _CACHE["nc"]

    in_maps, const_row = _prep_inputs(x, w_qkv, b_qkv, w_out, b_out)
    res = run_bass_kernel_spmd(nc, in_maps, core_ids=list(range(NCORES)))
    LAST_RESULTS = res

    out = np.zeros((T, E), dtype=np.float32)
    for r in res.results:
        out += np.asarray(r["y"], dtype=np.float32)
    out += const_row[None, :].astype(np.float32)
    return out.reshape(1, T, E)

